# revision 19
# baseline (speedup 1.0000x reference)
"""Trainium2 Bass kernel for nn_Decoder (Bahdanau-attention LSTM decoder step).

Sharding (8 NeuronCores):
  - Attention + 2-layer LSTM cell: data-parallel over batch (8 batches/core).
  - fc_out (vocab 50257) + embedding rows: sharded over vocab dim.
  - One small AllGather of out_cat^T [2048, 8] -> [16384, 8] joins the phases.

Per core the heavy work is:
  - stream its enc slice [8, 1024, 2048] once (f-major host layout),
    energy matmul on PE, tanh+exp on ACT, context accumulation via fused
    DVE tensor_tensor_reduce (softmax without max-subtraction: scores are
    bounded by ||v||_1 ~ 8 so exp() is safe, and masked scores get -1e10
    which underflows exp to exactly 0).
  - fc matmul over its padded vocab slice [2048, 6656].
"""

import os
import sys

import numpy as np

for _p in ("/opt/trn_rl_repo", "/opt/trn_rl_repo/concourse"):
    if _p not in sys.path and os.path.isdir(_p):
        sys.path.insert(0, _p)

import ml_dtypes

# ---------------------------------------------------------------- dims
VOCAB = 50257
EMB = 512
ENC2 = 1024
DEC = 512
ATTN = 512
B = 64
S = 2048
NCORES = 8

BF16 = ml_dtypes.bfloat16


def full_dims():
    return dict(
        VOCAB=VOCAB, EMB=EMB, ENC2=ENC2, DEC=DEC, ATTN=ATTN, B=B, S=S,
        NCORES=NCORES, S_T=512, V_TILE=512,
    )


def derived(d):
    g = dict(d)
    g["NB"] = d["B"] // d["NCORES"]          # local batches
    g["NST"] = d["S"] // d["S_T"]            # s tiles
    g["NF"] = d["ENC2"] // 128               # enc feature chunks
    g["NM"] = d["ATTN"] // 128               # attn chunks
    g["KH"] = d["DEC"] // 128                # dec feature chunks
    g["KE"] = d["EMB"] // 128                # emb chunks
    g["XF"] = d["EMB"] + d["ENC2"]
    g["KX"] = g["XF"] // 128
    g["F2"] = d["DEC"] + d["ENC2"] + d["EMB"]
    g["KF2"] = g["F2"] // 128
    g["V_STRIDE"] = -(-d["VOCAB"] // d["NCORES"])           # ceil
    nvt = -(-g["V_STRIDE"] // d["V_TILE"])
    g["NVT"] = nvt
    g["V_PAD"] = nvt * d["V_TILE"]
    return g


# dtype config: "bf16" (fast; bf16 data path) or "f32r" (fp32 data, fp32r matmuls)
CONFIG = os.environ.get("DEC_KERNEL_DTYPE", "bf16")


# ---------------------------------------------------------------- builder
def build(tc, g, cfg):
    import concourse.bass as bass
    import concourse.mybir as mybir
    from concourse.masks import make_identity

    nc = tc.nc
    f32 = mybir.dt.float32
    bf16 = mybir.dt.bfloat16
    f32r = mybir.dt.float32r

    mm16 = cfg == "bf16"
    edt = bf16 if mm16 else f32      # enc / energy / attention-weight dtype
    wdt = bf16 if mm16 else f32      # weight dtype (lstm/fc/attn proj)

    def cast(ap):
        # dtype handed to the PE: bf16 stays bf16; fp32 runs as fp32r
        if ap.dtype == f32:
            return ap.bitcast(f32r)
        return ap

    NB, NST, NF, NM = g["NB"], g["NST"], g["NF"], g["NM"]
    KH, KE, KX, KF2, F2 = g["KH"], g["KE"], g["KX"], g["KF2"], g["F2"]
    S_T, NVT, V_PAD, V_TILE = g["S_T"], g["NVT"], g["V_PAD"], g["V_TILE"]
    SS, DECD, ATTND, EMBD, ENC2D = g["S"], g["DEC"], g["ATTN"], g["EMB"], g["ENC2"]
    BD = g["B"]
    ncores = g["NCORES"]

    AF = mybir.ActivationFunctionType
    ALU = mybir.AluOpType
    AX = mybir.AxisListType

    # ----- external I/O -----
    enc_f = nc.dram_tensor("enc_f", [NB, ENC2D, SS], edt, kind="ExternalInput")
    wenc_t = nc.dram_tensor("wenc_t", [ENC2D, ATTND], edt, kind="ExternalInput")
    whid_t = nc.dram_tensor("whid_t", [DECD, ATTND], wdt, kind="ExternalInput")
    v_h = nc.dram_tensor("v_h", [128, NM], edt, kind="ExternalInput")
    attnb_h = nc.dram_tensor("attnb_h", [128, NM], f32, kind="ExternalInput")
    mask_off = nc.dram_tensor("mask_off", [1, NB * SS], f32, kind="ExternalInput")
    embT = nc.dram_tensor("embT", [EMBD, NB], f32, kind="ExternalInput")
    h0T0 = nc.dram_tensor("h0T0", [DECD, NB], wdt, kind="ExternalInput")
    h0T1 = nc.dram_tensor("h0T1", [DECD, NB], wdt, kind="ExternalInput")
    c0_0 = nc.dram_tensor("c0_0", [NB, DECD], f32, kind="ExternalInput")
    c0_1 = nc.dram_tensor("c0_1", [NB, DECD], f32, kind="ExternalInput")
    wih0 = nc.dram_tensor("wih0", [g["XF"], 4 * DECD], wdt, kind="ExternalInput")
    whh0 = nc.dram_tensor("whh0", [DECD, 4 * DECD], wdt, kind="ExternalInput")
    wih1 = nc.dram_tensor("wih1", [DECD, 4 * DECD], wdt, kind="ExternalInput")
    whh1 = nc.dram_tensor("whh1", [DECD, 4 * DECD], wdt, kind="ExternalInput")
    b0 = nc.dram_tensor("b0", [2, 4 * DECD], f32, kind="ExternalInput")
    b1 = nc.dram_tensor("b1", [2, 4 * DECD], f32, kind="ExternalInput")
    fcw = nc.dram_tensor("fcw", [F2, V_PAD], wdt, kind="ExternalInput")
    fcb = nc.dram_tensor("fcb", [1, V_PAD], f32, kind="ExternalInput")

    logits_o = nc.dram_tensor("logits", [BD, V_PAD], f32, kind="ExternalOutput")
    attn_o = nc.dram_tensor("attn", [NB, SS], f32, kind="ExternalOutput")
    h_o = nc.dram_tensor("h_new", [2, NB, DECD], f32, kind="ExternalOutput")
    c_o = nc.dram_tensor("c_new", [2, NB, DECD], f32, kind="ExternalOutput")

    rg = [list(range(ncores))]

    from contextlib import ExitStack

    est = ExitStack()
    with est:
        const = est.enter_context(tc.tile_pool(name="const", bufs=1))
        dram = est.enter_context(tc.tile_pool(name="dram", bufs=1, space="DRAM"))

        # ----- resident constants -----
        ident = const.tile([128, 128], f32)
        make_identity(nc, ident[:])

        wenc_sb = const.tile([128, NF * ATTND], edt)
        nc.sync.dma_start(
            out=wenc_sb[:].rearrange("p (c a) -> p c a", c=NF),
            in_=wenc_t.ap().rearrange("(c p) a -> p c a", p=128),
        )
        whid_sb = const.tile([128, KH * ATTND], wdt)
        nc.sync.dma_start(
            out=whid_sb[:].rearrange("p (c a) -> p c a", c=KH),
            in_=whid_t.ap().rearrange("(c p) a -> p c a", p=128),
        )
        v_sb = const.tile([128, NM], edt)
        nc.sync.dma_start(out=v_sb[:], in_=v_h.ap())
        attnb_sb = const.tile([128, NM], f32)
        nc.sync.dma_start(out=attnb_sb[:], in_=attnb_h.ap())

        h0T0_sb = const.tile([128, KH * NB], wdt)
        nc.sync.dma_start(
            out=h0T0_sb[:].rearrange("p (c b) -> p c b", c=KH),
            in_=h0T0.ap().rearrange("(c p) b -> p c b", p=128),
        )
        h0T1_sb = const.tile([128, KH * NB], wdt)
        nc.sync.dma_start(
            out=h0T1_sb[:].rearrange("p (c b) -> p c b", c=KH),
            in_=h0T1.ap().rearrange("(c p) b -> p c b", p=128),
        )
        c00_sb = const.tile([NB, DECD], f32)
        nc.sync.dma_start(out=c00_sb[:], in_=c0_0.ap())
        c01_sb = const.tile([NB, DECD], f32)
        nc.sync.dma_start(out=c01_sb[:], in_=c0_1.ap())
        # per-layer gate bias: b_ih + b_hh summed on partition 0, then
        # broadcast to the NB batch partitions
        bias_bc = []
        for name, src in (("b0", b0), ("b1", b1)):
            ba = const.tile([1, 4 * DECD], f32, name=f"{name}a")
            bb = const.tile([1, 4 * DECD], f32, name=f"{name}b")
            nc.sync.dma_start(out=ba[:], in_=src.ap()[0:1, :])
            nc.sync.dma_start(out=bb[:], in_=src.ap()[1:2, :])
            bs = const.tile([1, 4 * DECD], f32, name=f"{name}s")
            nc.vector.tensor_add(bs[:], ba[:], bb[:])
            bc = const.tile([NB, 4 * DECD], f32, name=f"{name}c")
            nc.gpsimd.partition_broadcast(bc[:], bs[:])
            bias_bc.append(bc)


        # x^T tile: [emb chunks | ctx chunks] columns of NB each (f32 master)
        xT = const.tile([128, KX * NB], f32)
        nc.sync.dma_start(
            out=xT[:, 0:KE * NB].rearrange("p (c b) -> p c b", c=KE),
            in_=embT.ap().rearrange("(c p) b -> p c b", p=128),
        )

        inv8 = const.tile([1, NB], f32)
        inv_bc = const.tile([128, NB], f32)
        # ctx partial sums: column (c*NB + b)*NST + st
        accN = const.tile([128, NF * NB * NST], f32)

        # ----- q = W_hid @ h0[-1] + attn_b  -> [ATTN(part-chunks), NB] -----
        with tc.tile_pool(name="qp", bufs=1, space="PSUM") as qpp:
            q_ps = qpp.tile([128, NM * NB], f32)
            q_sb = const.tile([128, NM * NB], f32)
            for m in range(NM):
                for k in range(KH):
                    nc.tensor.matmul(
                        q_ps[:, m * NB:(m + 1) * NB],
                        lhsT=whid_sb[:, k * ATTND + m * 128: k * ATTND + (m + 1) * 128],
                        rhs=h0T1_sb[:, k * NB:(k + 1) * NB],
                        start=(k == 0), stop=(k == KH - 1),
                    )
                nc.vector.tensor_scalar_add(
                    q_sb[:, m * NB:(m + 1) * NB], q_ps[:, m * NB:(m + 1) * NB],
                    attnb_sb[:, m:m + 1],
                )

        # ----- attention main loop -----
        with (
            tc.tile_pool(name="encp", bufs=3) as encp,
            tc.tile_pool(name="ep", bufs=2, space="PSUM") as epp,
            tc.tile_pool(name="sp", bufs=2, space="PSUM") as spp,
            tc.tile_pool(name="esb", bufs=3) as esbp,
            tc.tile_pool(name="scp", bufs=2) as scp,
            tc.tile_pool(name="mkp", bufs=2) as mkp,
            tc.tile_pool(name="wfp", bufs=2) as wfp,
            tc.tile_pool(name="wbp", bufs=2) as wbp,
            tc.tile_pool(name="w16p", bufs=2) as w16p,
            tc.tile_pool(name="ttrs", bufs=2) as ttrp,
        ):
            for b in range(NB):
                wrow = wfp.tile([1, SS], f32, tag="wrow")
                for st in range(NST):
                    enc_t = encp.tile([128, NF * S_T], edt, tag="enc")
                    nc.sync.dma_start(
                        out=enc_t[:].rearrange("p (c s) -> p c s", c=NF),
                        in_=enc_f.ap()[b].rearrange("(c p) s -> p c s", p=128)[
                            :, :, st * S_T:(st + 1) * S_T],
                    )
                    s_ps = spp.tile([1, S_T], f32, tag="sps")
                    for m in range(NM):
                        e_ps = epp.tile([128, S_T], f32, tag="eps")
                        for c in range(NF):
                            nc.tensor.matmul(
                                e_ps[:],
                                lhsT=cast(wenc_sb[:, c * ATTND + m * 128:
                                                  c * ATTND + (m + 1) * 128]),
                                rhs=cast(enc_t[:, c * S_T:(c + 1) * S_T]),
                                start=(c == 0), stop=(c == NF - 1),
                            )
                        e_sb = esbp.tile([128, S_T], edt, tag="esb")
                        nc.scalar.activation(
                            e_sb[:], e_ps[:], AF.Tanh,
                            bias=q_sb[:, m * NB + b: m * NB + b + 1],
                        )
                        nc.tensor.matmul(
                            s_ps[:], lhsT=cast(v_sb[:, m:m + 1]), rhs=cast(e_sb[:]),
                            start=(m == 0), stop=(m == NM - 1),
                        )
                    mk = mkp.tile([1, S_T], f32, tag="mk")
                    nc.sync.dma_start(
                        out=mk[:],
                        in_=mask_off.ap()[0:1, b * SS + st * S_T:
                                          b * SS + (st + 1) * S_T],
                    )
                    sc_sb = scp.tile([1, S_T], f32, tag="sc")
                    nc.vector.tensor_add(sc_sb[:], s_ps[:], mk[:])
                    wsl = wrow[0:1, st * S_T:(st + 1) * S_T]
                    nc.scalar.activation(wsl, sc_sb[:], AF.Exp)
                    if mm16:
                        w16 = w16p.tile([1, S_T], edt, tag="w16")
                        nc.vector.tensor_copy(w16[:], wsl)
                        wsrc = w16[:]
                    else:
                        wsrc = wsl
                    wb = wbp.tile([128, S_T], edt, tag="wb")
                    nc.gpsimd.partition_broadcast(wb[:], wsrc)
                    for c in range(NF):
                        col = accN[:, (c * NB + b) * NST + st:
                                   (c * NB + b) * NST + st + 1]
                        scr = ttrp.tile([128, S_T], edt, tag="scr")
                        nc.vector.tensor_mul(
                            scr[:], enc_t[:, c * S_T:(c + 1) * S_T], wb[:],
                        )
                        nc.vector.reduce_sum(col, scr[:], axis=AX.X)
                # per-b softmax denominator + attn row out
                nc.vector.reduce_sum(inv8[0:1, b:b + 1], wrow[:], axis=AX.X)
                nc.vector.reciprocal(inv8[0:1, b:b + 1], inv8[0:1, b:b + 1])
                nc.vector.tensor_scalar_mul(
                    wrow[:], wrow[:], inv8[0:1, b:b + 1],
                )
                nc.sync.dma_start(out=attn_o.ap()[b:b + 1, :], in_=wrow[:])

        # fold the NST partials into ctx columns of xT, then normalize
        nc.vector.reduce_sum(
            xT[:, KE * NB:(KE + NF) * NB],
            accN[:].rearrange("p (cb t) -> p cb t", t=NST), axis=AX.X,
        )
        nc.gpsimd.partition_broadcast(inv_bc[:], inv8[:])
        for c in range(NF):
            nc.vector.tensor_mul(
                xT[:, (KE + c) * NB:(KE + c + 1) * NB],
                xT[:, (KE + c) * NB:(KE + c + 1) * NB],
                inv_bc[:],
            )

        # ----- LSTM (2 layers, single step) -----
        if mm16:
            xT16 = const.tile([128, KX * NB], wdt)
            nc.vector.tensor_copy(xT16[:], xT[:])
            xin = xT16
        else:
            xin = xT

        def lstm_layer(lhs_chunks, w_x, w_h, bias_bc_t, c_prev, h_out_sb, c_out_sb,
                       pools):
            gpp, grhs, gsb, tvp = pools
            nk = len(lhs_chunks)
            gates = []
            for n in range(4):
                g_ps = gpp.tile([NB, DECD], f32, tag="gps")
                for idx, (lh, src, k) in enumerate(lhs_chunks):
                    rt = grhs.tile([128, DECD], wdt, tag="grhs")
                    nc.sync.dma_start(
                        out=rt[:],
                        in_=src.ap()[k * 128:(k + 1) * 128,
                                     n * DECD:(n + 1) * DECD],
                    )
                    nc.tensor.matmul(g_ps[:], lhsT=cast(lh), rhs=cast(rt[:]),
                                     start=(idx == 0), stop=(idx == nk - 1))
                gb = gsb.tile([NB, DECD], f32, tag="gsb")
                nc.vector.tensor_add(
                    gb[:], g_ps[:], bias_bc_t[:, n * DECD:(n + 1) * DECD])
                gt = gsb.tile([NB, DECD], f32, tag="gsb")
                nc.scalar.activation(gt[:], gb[:],
                                     AF.Tanh if n == 2 else AF.Sigmoid)
                gates.append(gt)
            t1 = tvp.tile([NB, DECD], f32, tag="tv")
            nc.vector.tensor_mul(t1[:], gates[1][:], c_prev[:])
            t2 = tvp.tile([NB, DECD], f32, tag="tv")
            nc.vector.tensor_mul(t2[:], gates[0][:], gates[2][:])
            nc.vector.tensor_add(c_out_sb[:], t1[:], t2[:])
            tc1 = tvp.tile([NB, DECD], f32, tag="tv")
            nc.scalar.activation(tc1[:], c_out_sb[:], AF.Tanh)
            nc.vector.tensor_mul(h_out_sb[:], gates[3][:], tc1[:])

        h1_sb = const.tile([NB, DECD], f32)
        c1_sb = const.tile([NB, DECD], f32)
        h2_sb = const.tile([NB, DECD], f32)
        c2_sb = const.tile([NB, DECD], f32)
        h1T_sb = const.tile([128, KH * NB], wdt)
        h2T_sb = const.tile([128, KH * NB], f32)

        with (
            tc.tile_pool(name="gp", bufs=2, space="PSUM") as gpp,
            tc.tile_pool(name="grhs", bufs=3) as grhs,
            tc.tile_pool(name="gsb", bufs=8) as gsb,
            tc.tile_pool(name="tv", bufs=4) as tvp,
            tc.tile_pool(name="tpp", bufs=2, space="PSUM") as tpp,
        ):
            pools = (gpp, grhs, gsb, tvp)
            chunks0 = [(xin[:, k * NB:(k + 1) * NB], wih0, k) for k in range(KX)]
            chunks0 += [(h0T0_sb[:, k * NB:(k + 1) * NB], whh0, k) for k in range(KH)]
            lstm_layer(chunks0, wih0, whh0, bias_bc[0], c00_sb, h1_sb, c1_sb, pools)
            nc.sync.dma_start(out=h_o.ap()[0], in_=h1_sb[:])
            nc.sync.dma_start(out=c_o.ap()[0], in_=c1_sb[:])
            for k in range(KH):
                tp = tpp.tile([128, NB], f32, tag="tp")
                nc.tensor.transpose(
                    tp[:], h1_sb[:, k * 128:(k + 1) * 128], ident[0:NB, 0:NB]
                )
                nc.scalar.activation(h1T_sb[:, k * NB:(k + 1) * NB], tp[:], AF.Copy)
            chunks1 = [(h1T_sb[:, k * NB:(k + 1) * NB], wih1, k) for k in range(KH)]
            chunks1 += [(h0T1_sb[:, k * NB:(k + 1) * NB], whh1, k) for k in range(KH)]
            lstm_layer(chunks1, wih1, whh1, bias_bc[1], c01_sb, h2_sb, c2_sb, pools)
            nc.sync.dma_start(out=h_o.ap()[1], in_=h2_sb[:])
            nc.sync.dma_start(out=c_o.ap()[1], in_=c2_sb[:])
            for k in range(KH):
                tp = tpp.tile([128, NB], f32, tag="tp")
                nc.tensor.transpose(
                    tp[:], h2_sb[:, k * 128:(k + 1) * 128], ident[0:NB, 0:NB]
                )
                nc.scalar.activation(h2T_sb[:, k * NB:(k + 1) * NB], tp[:], AF.Copy)

        # ----- out_cat^T -> AllGather -> xcat -----
        oc = dram.tile([F2, NB], f32)
        oc_ag = dram.tile([ncores * F2, NB], f32)
        nc.sync.dma_start(
            out=oc[0:DECD, :].rearrange("(c p) b -> p c b", p=128),
            in_=h2T_sb[:].rearrange("p (c b) -> p c b", c=KH),
        )
        nc.sync.dma_start(
            out=oc[DECD:DECD + ENC2D, :].rearrange("(c p) b -> p c b", p=128),
            in_=xT[:, KE * NB:(KE + NF) * NB].rearrange("p (c b) -> p c b", c=NF),
        )
        nc.sync.dma_start(
            out=oc[DECD + ENC2D:F2, :].rearrange("(c p) b -> p c b", p=128),
            in_=xT[:, 0:KE * NB].rearrange("p (c b) -> p c b", c=KE),
        )
        nc.gpsimd.collective_compute(
            "AllGather", mybir.AluOpType.bypass,
            replica_groups=rg, ins=[oc.opt()], outs=[oc_ag.opt()],
        )
        xcat = const.tile([128, KF2 * BD], f32)
        for r in range(ncores):
            nc.sync.dma_start(
                out=xcat[:].rearrange("p (c r b) -> p c r b", r=ncores, b=NB)[
                    :, :, r, :],
                in_=oc_ag[:][r * F2:(r + 1) * F2, :].rearrange(
                    "(c p) b -> p c b", p=128),
            )
        if mm16:
            xcat16 = const.tile([128, KF2 * BD], wdt)
            nc.vector.tensor_copy(xcat16[:], xcat[:])
            xc = xcat16
        else:
            xc = xcat

        # ----- fc_out over local vocab slice -----
        with (
            tc.tile_pool(name="lp", bufs=2, space="PSUM") as lpp,
            tc.tile_pool(name="fwp", bufs=8) as fwp,
            tc.tile_pool(name="fbp", bufs=2) as fbp,
            tc.tile_pool(name="lsb", bufs=2) as lsbp,
        ):
            for n in range(NVT):
                l_ps = lpp.tile([BD, V_TILE], f32, tag="lps")
                fcb_t = fbp.tile([1, V_TILE], f32, tag="fcb")
                nc.sync.dma_start(
                    out=fcb_t[:],
                    in_=fcb.ap()[0:1, n * V_TILE:(n + 1) * V_TILE],
                )
                fcb_bc = fbp.tile([BD, V_TILE], f32, tag="fcbb")
                nc.gpsimd.partition_broadcast(fcb_bc[:], fcb_t[:])
                for k in range(KF2):
                    fw = fwp.tile([128, V_TILE], wdt, tag="fw")
                    nc.sync.dma_start(
                        out=fw[:],
                        in_=fcw.ap()[k * 128:(k + 1) * 128,
                                     n * V_TILE:(n + 1) * V_TILE],
                    )
                    nc.tensor.matmul(
                        l_ps[:], lhsT=cast(xc[:, k * BD:(k + 1) * BD]),
                        rhs=cast(fw[:]), start=(k == 0), stop=(k == KF2 - 1),
                    )
                l_sb = lsbp.tile([BD, V_TILE], f32, tag="lsb")
                nc.vector.tensor_add(l_sb[:], l_ps[:], fcb_bc[:])
                nc.sync.dma_start(
                    out=logits_o.ap()[:, n * V_TILE:(n + 1) * V_TILE], in_=l_sb[:]
                )

    return nc


def build_kernel(dims=None, cfg=None):
    import concourse.tile as tile
    from concourse import bacc

    g = derived(dims or full_dims())
    cfg = cfg or CONFIG
    nc = bacc.Bacc(
        "TRN2", target_bir_lowering=False, debug=False,
        num_devices=g["NCORES"],
    )
    with tile.TileContext(nc) as tc:
        build(tc, g, cfg)
    nc.compile()
    return nc, g


# ---------------------------------------------------------------- host prep
def prep_core_inputs(r, inp, g, cfg):
    """Shard + lay out the full inputs for core r (pure data movement +
    dtype conversion; the one arithmetic preprocessing is turning the 0/1
    int mask into the additive -1e10 offset the reference applies)."""
    mm16 = cfg == "bf16"
    edt = BF16 if mm16 else np.float32
    wdt = BF16 if mm16 else np.float32
    NB, NM, V_PAD = g["NB"], g["NM"], g["V_PAD"]
    F2, VS = g["F2"], g["V_STRIDE"]
    bs = slice(r * NB, (r + 1) * NB)

    enc = np.asarray(inp["encoder_outputs"], np.float32)
    enc_f = np.ascontiguousarray(enc[:, bs, :].transpose(1, 2, 0)).astype(edt)

    attn_W = np.asarray(inp["attn_W"], np.float32)
    wenc_t = np.ascontiguousarray(attn_W[:, g["DEC"]:].T).astype(edt)
    whid_t = np.ascontiguousarray(attn_W[:, :g["DEC"]].T).astype(wdt)
    v_h = np.ascontiguousarray(
        np.asarray(inp["v_w"], np.float32).reshape(NM, 128).T).astype(edt)
    attnb_h = np.ascontiguousarray(
        np.asarray(inp["attn_b"], np.float32).reshape(NM, 128).T)

    mask = np.asarray(inp["mask"])[bs]
    mask_off = ((mask != 0).astype(np.float32) - 1.0) * 1e10
    mask_off = np.ascontiguousarray(mask_off.reshape(1, -1))

    tok = np.asarray(inp["input_tok"]).astype(np.int64)[bs]
    embT = np.ascontiguousarray(
        np.asarray(inp["emb"], np.float32)[tok].T)

    h0 = np.asarray(inp["h0"], np.float32)
    c0 = np.asarray(inp["c0"], np.float32)
    h0T0 = np.ascontiguousarray(h0[0][bs].T).astype(wdt)
    h0T1 = np.ascontiguousarray(h0[1][bs].T).astype(wdt)
    c0_0 = np.ascontiguousarray(c0[0][bs])
    c0_1 = np.ascontiguousarray(c0[1][bs])

    wih0 = np.ascontiguousarray(np.asarray(inp["W_ih0"], np.float32).T).astype(wdt)
    whh0 = np.ascontiguousarray(np.asarray(inp["W_hh0"], np.float32).T).astype(wdt)
    wih1 = np.ascontiguousarray(np.asarray(inp["W_ih1"], np.float32).T).astype(wdt)
    whh1 = np.ascontiguousarray(np.asarray(inp["W_hh1"], np.float32).T).astype(wdt)
    b0 = np.ascontiguousarray(
        np.stack([np.asarray(inp["b_ih0"], np.float32),
                  np.asarray(inp["b_hh0"], np.float32)]))
    b1 = np.ascontiguousarray(
        np.stack([np.asarray(inp["b_ih1"], np.float32),
                  np.asarray(inp["b_hh1"], np.float32)]))

    fc_W = np.asarray(inp["fc_W"], np.float32)
    lo = r * VS
    hi = min(fc_W.shape[0], (r + 1) * VS)
    cnt = hi - lo
    fcw = np.zeros((F2, V_PAD), wdt)
    fcw[:, :cnt] = fc_W[lo:hi].T.astype(wdt)
    fcb = np.zeros((1, V_PAD), np.float32)
    fcb[0, :cnt] = np.asarray(inp["fc_b"], np.float32)[lo:hi]

    return dict(
        enc_f=enc_f, wenc_t=wenc_t, whid_t=whid_t, v_h=v_h, attnb_h=attnb_h,
        mask_off=mask_off, embT=embT, h0T0=h0T0, h0T1=h0T1, c0_0=c0_0,
        c0_1=c0_1, wih0=wih0, whh0=whh0, wih1=wih1, whh1=whh1, b0=b0, b1=b1,
        fcw=fcw, fcb=fcb,
    ), cnt


_CACHE = {}
last_results = None


def kernel(**inputs):
    """Full inputs in, full outputs out; 8-core SPMD Bass kernel inside."""
    global last_results
    from concourse.bass_utils import run_bass_kernel_spmd

    g_key = ("full", CONFIG)
    if g_key not in _CACHE:
        _CACHE[g_key] = build_kernel()
    nc, g = _CACHE[g_key]

    in_maps = []
    counts = []
    for r in range(g["NCORES"]):
        m, cnt = prep_core_inputs(r, inputs, g, CONFIG)
        in_maps.append(m)
        counts.append(cnt)

    trace = bool(os.environ.get("DEC_KERNEL_TRACE"))
    res = run_bass_kernel_spmd(
        nc, in_maps, list(range(g["NCORES"])), trace=trace,
    )
    last_results = res
    if res.exec_time_ns is not None:
        print(f"HW exec time: {res.exec_time_ns} ns")
    results = res.results

    pred = np.concatenate(
        [results[r]["logits"][:, :counts[r]] for r in range(g["NCORES"])], axis=1
    ).astype(np.float32)
    h_new = np.concatenate(
        [results[r]["h_new"] for r in range(g["NCORES"])], axis=1
    ).astype(np.float32)
    c_new = np.concatenate(
        [results[r]["c_new"] for r in range(g["NCORES"])], axis=1
    ).astype(np.float32)
    attn = np.concatenate(
        [results[r]["attn"] for r in range(g["NCORES"])], axis=0
    ).astype(np.float32)
    return pred, h_new, c_new, attn
